# revision 37
# baseline (speedup 1.0000x reference)
"""Masked multi-head attention (B=32, N=512, E=512, H=8) on 8 Trainium2 cores.

Sharding: data-parallel over batch (4 batches per core); weights and the
attention mask are replicated. All layout transforms (weight transposes,
x transpose, mask transforms, bias broadcast, bf16/fp8 casts) are host-side
numpy, so the device kernel is pure matmul/softmax work.

Per-core pipeline (per batch; scores/PV math bf16, Q/K proj fp8, psum fp32):
  qT = Wq8.T @ x8 (+bq)      fp8e4m3 DoubleRow matmuls (2 k-chunks/pass),
  kT = Wk8.T @ x8 (+bk)      e-major [e_out, n]; bias via ScalarE [P,1] add
  v  = xT.T @ WvT (+bv)      bf16, n-major, ones column appended per head so
                             the softmax denominator falls out of the P@V mm
  per head pair:
    psum = kT_h.T @ qT_h     scores transposed [k, q], two banks per chunk
    P    = exp(psum / 8)     one ACTIVATE per 2 banks, scale fused, bf16 out
    P   *= adj.T             VectorE mask multiply (scores are small, so no
                             max-subtraction is needed)
  P@V: per (pair, qi-pair) ONE psum bank holds 4 groups [qi&1, hh] of
    [q, v_h | 1] at 128-col offsets (col 64 = denominator); normalization is
    a strided reciprocal + one broadcast 4D multiply per bank (24 DVE
    ops/batch vs 256); the last pair's P@V+norm defer into the next batch
    to fill the boundary bubble.
  oT = DMA-transpose(o)      per-pair, right after each norm (spreads HWDGE)
  out = oT.T @ WoT (+bo via a K=1 ones-row matmul); psum evacuated by DVE;
                             interleaved into the NEXT batch's attention
  Mask multiplies split ~3:1 between DVE and GpSimd; q/k evacuations are
  DVE TensorScalarPtr ops so ScalarE runs exp-only (no act-table swaps).
"""

import numpy as np

import concourse.bass as bass
import concourse.tile as tile
from concourse import bacc, mybir
import concourse.bass_utils as bass_utils

N_CORES = 8
B, N, E, H = 32, 512, 512, 8
DH = E // H  # 64
BPC = B // N_CORES  # batches per core
P = 128
NT = N // P  # 4 tiles along sequence
ET = E // P  # 4 tiles along embedding
FP32 = mybir.dt.float32
BF16 = mybir.dt.bfloat16
FP8 = mybir.dt.float8e4
DR = mybir.MatmulPerfMode.DoubleRow
AF = mybir.ActivationFunctionType


# tunable knobs
CFG = {
    "xt": 2, "qt": 2, "kt": 2, "vx": 2, "pt": 6, "ot": 3, "out": 4,
    "small": 8, "scores": 2, "ps": 2, "pso": 2,
    "qk_fp8": True, "dma_q": "sync", "trans_q": "sync", "out_q": "sync",
    "pack_norm": True, "defer_pv": True, "pool_mask_frac": 1, "evac_q": "vector", "evac_k": "vector",
}


def build_nc(loop_iters=1):
    nc = bacc.Bacc("TRN2", target_bir_lowering=False, debug=False,
                   num_devices=N_CORES)

    xT_d = nc.dram_tensor("xT", [BPC, E, N], BF16, kind="ExternalInput")
    x8_d = (nc.dram_tensor("xT8", [BPC, E, N], FP8, kind="ExternalInput")
            if CFG["qk_fp8"] else None)
    qdt = FP8 if CFG["qk_fp8"] else BF16
    wq_d = nc.dram_tensor("Wq8", [E, E], qdt, kind="ExternalInput")
    wk_d = nc.dram_tensor("Wk8", [E, E], qdt, kind="ExternalInput")
    wv_d = nc.dram_tensor("WvT", [E, E], BF16, kind="ExternalInput")
    wo_d = nc.dram_tensor("WoT", [E, E], BF16, kind="ExternalInput")
    bq_d = nc.dram_tensor("bqT", [P, ET], FP32, kind="ExternalInput")
    bk_d = nc.dram_tensor("bkT", [P, ET], FP32, kind="ExternalInput")
    bv_d = nc.dram_tensor("bvB", [P, E], FP32, kind="ExternalInput")
    bo_d = nc.dram_tensor("boB", [P, E], FP32, kind="ExternalInput")
    adj_d = nc.dram_tensor("adjT", [N, N], BF16, kind="ExternalInput")
    bo16_d = nc.dram_tensor("bo16", [1, E], BF16, kind="ExternalInput")
    out_d = nc.dram_tensor("out", [BPC, N, E], FP32, kind="ExternalOutput")

    with tile.TileContext(nc) as tc:
        with (
            tc.tile_pool(name="persist", bufs=1) as persist,
            tc.tile_pool(name="xt", bufs=CFG["xt"]) as xt_pool,
            tc.tile_pool(name="x8", bufs=CFG["xt"]) as x8_pool,
            tc.tile_pool(name="qt", bufs=CFG["qt"]) as qt_pool,
            tc.tile_pool(name="kt", bufs=CFG["kt"]) as kt_pool,
            tc.tile_pool(name="vx", bufs=CFG["vx"]) as vx_pool,
            tc.tile_pool(name="pt", bufs=CFG["pt"]) as pt_pool,
            tc.tile_pool(name="osb", bufs=2) as o_pool,
            tc.tile_pool(name="otsb", bufs=CFG["ot"]) as ot_pool,
            tc.tile_pool(name="outsb", bufs=CFG["out"]) as out_pool,
            tc.tile_pool(name="small", bufs=CFG["small"]) as small_pool,
            tc.tile_pool(name="ps_big", bufs=CFG["scores"], space="PSUM") as ps_big,
            tc.tile_pool(name="ps_small", bufs=CFG["ps"], space="PSUM") as ps_small,
            tc.tile_pool(name="ps_o", bufs=CFG["pso"], space="PSUM") as ps_o_pool,
        ):
            # ---- persistent tensors (replicated weights / mask / biases)
            wq_sb = persist.tile([P, ET, E], qdt)
            nc.sync.dma_start(wq_sb[:], wq_d.ap().rearrange("(c p) e -> p c e", p=P))
            bq_sb = persist.tile([P, ET], FP32)
            nc.sync.dma_start(bq_sb[:], bq_d.ap())
            wk_sb = persist.tile([P, ET, E], qdt)
            nc.sync.dma_start(wk_sb[:], wk_d.ap().rearrange("(c p) e -> p c e", p=P))
            bk_sb = persist.tile([P, ET], FP32)
            nc.sync.dma_start(bk_sb[:], bk_d.ap())
            wv_sb = persist.tile([P, ET, E], BF16)
            bv_sb = persist.tile([P, E], FP32)
            adj_sb = persist.tile([P, NT, N], BF16)
            wo_sb = persist.tile([P, ET, E], BF16)
            bo_sb = persist.tile([P, E], FP32)
            ones1 = persist.tile([1, P], BF16)
            nc.vector.memset(ones1[:], 1.0)
            bo1 = persist.tile([1, E], BF16)

            def load_persist2(loop_iters=loop_iters):
                nc.scalar.dma_start(
                    adj_sb[:],
                    adj_d.ap().rearrange("(c p) q -> p c q", p=P))
                nc.scalar.dma_start(
                    wv_sb[:], wv_d.ap().rearrange("(c p) e -> p c e", p=P))
                nc.scalar.dma_start(bv_sb[:], bv_d.ap())
                nc.scalar.dma_start(
                    wo_sb[:], wo_d.ap().rearrange("(c p) e -> p c e", p=P))
                nc.scalar.dma_start(bo_sb[:], bo_d.ap())
                nc.scalar.dma_start(bo1[:], bo16_d.ap())

            import contextlib
            if loop_iters > 1:
                load_persist2()
            loop_cm = (tc.For_i(0, loop_iters, 1) if loop_iters > 1
                       else contextlib.nullcontext())
            with loop_cm:
                body(nc, tc, locals())

    nc.compile()
    return nc


def body(nc, tc, env):
    (xT_d, x8_d, out_d, wq_sb, wk_sb, wv_sb, wo_sb, adj_sb, bq_sb, bk_sb,
     bv_sb, bo_sb, ones1, bo1) = (env[k] for k in (
         "xT_d", "x8_d", "out_d", "wq_sb", "wk_sb", "wv_sb", "wo_sb",
         "adj_sb", "bq_sb", "bk_sb", "bv_sb", "bo_sb", "ones1", "bo1"))

    (xt_pool, x8_pool, qt_pool, kt_pool, vx_pool, pt_pool, o_pool, ot_pool,
     out_pool, small_pool, ps_big, ps_small, ps_o_pool) = (env[k] for k in (
         "xt_pool", "x8_pool", "qt_pool", "kt_pool", "vx_pool", "pt_pool",
         "o_pool", "ot_pool", "out_pool", "small_pool", "ps_big", "ps_small",
         "ps_o_pool"))
    MUL = mybir.AluOpType.mult
    env["pending_final"] = None
    env["pending_pv"] = None
    dma_q = getattr(nc, CFG["dma_q"])
    trans_q = getattr(nc, CFG["trans_q"])
    out_q = getattr(nc, CFG["out_q"])

    def issue_o_head(st, hp, hh):
        # P@V for head h = 2*hp + hh; psum bank per (hp, qi-pair) holds
        # four 65-col groups [qi&1, hh] at 128-col offsets
        bb, pts, vx, o_sb, pso_live, ot = st
        h = 2 * hp + hh
        for qi in range(NT):
            qi2 = qi // 2
            if hh == 0 and qi % 2 == 0:
                pso_live[qi2] = env["ps_o_pool"].tile(
                    [P, 4, DH + 1], FP32, tag="pso",
                    padded_shape=[P, 4, P], name=f"pso_{bb}_{hp}_{qi2}")
            ps_o = pso_live[qi2]
            j = (qi % 2) * 2 + hh
            for kt in range(NT):
                nc.tensor.matmul(
                    ps_o[:, j, 0:DH + 1],
                    pts[hh][:, kt * N + qi * P:kt * N + qi * P + P],
                    vx[:, kt, h, :],
                    start=(kt == 0), stop=(kt == NT - 1))

    def issue_norm(st, hp):
        # o[q, h*64:(h+1)*64] = num / den for the pair's 4 (qi&1, hh)
        # groups per bank: strided reciprocal + broadcast multiply
        bb, pts, vx, o_sb, pso_live, ot = st
        for qi2 in range(2):
            ps_o = pso_live.pop(qi2)
            rc = env["small_pool"].tile([P, 4, 1], FP32, tag="rc",
                                        name=f"rc_{bb}_{hp}_{qi2}")
            nc.vector.reciprocal(rc[:], ps_o[:, :, DH:DH + 1])
            nc.vector.tensor_tensor(
                o_sb[:, 2 * qi2:2 * qi2 + 2,
                     hp * 2 * DH:(hp + 1) * 2 * DH].rearrange(
                    "p q (c d) -> p q c d", d=DH),
                ps_o[:, :, 0:DH].rearrange("p (a c) d -> p a c d", c=2),
                rc.rearrange("p (a c) d -> p a c d", c=2).broadcast_to(
                    [P, 2, 2, DH]),
                op=MUL)
        for nt in range(NT):
            trans_q.dma_start_transpose(
                ot[:, hp, nt * P:(nt + 1) * P],
                o_sb[:, nt, hp * P:(hp + 1) * P])

    def load_x(b):
        x8 = None
        if CFG["qk_fp8"]:
            x8 = x8_pool.tile([P, ET, N], FP8, name=f"x8_{b}")
            dma_q.dma_start(
                x8[:], x8_d.ap()[b].rearrange("(c p) n -> p c n", p=P))
        xt = xt_pool.tile([P, ET, N], BF16, name=f"xt_{b}")
        dma_q.dma_start(xt[:], xT_d.ap()[b].rearrange("(c p) n -> p c n", p=P))
        return xt, x8

    env["next_x"] = load_x(0)
    if env["loop_iters"] == 1:
        env["load_persist2"]()
    for b in range(BPC):
        xt, x8 = env["next_x"]

        # ---- q/k projections, e-major output (qT[e_out, n])
        qt = qt_pool.tile([P, ET, N], BF16)
        ktl = kt_pool.tile([P, ET, N], BF16)
        for t in range(ET):
            for w_sb, b_sb, dst in ((wq_sb, bq_sb, qt), (wk_sb, bk_sb, ktl)):
                ps = ps_small.tile([P, N], FP32, tag="ps")
                if CFG["qk_fp8"]:
                    for c in range(ET // 2):
                        nc.tensor.matmul(
                            ps[:],
                            w_sb[:, 2 * c:2 * c + 2, t * P:(t + 1) * P],
                            x8[:, 2 * c:2 * c + 2, :],
                            start=(c == 0), stop=(c == ET // 2 - 1),
                            perf_mode=DR)
                else:
                    for kc in range(ET):
                        nc.tensor.matmul(
                            ps[:], w_sb[:, kc, t * P:(t + 1) * P],
                            xt[:, kc, :],
                            start=(kc == 0), stop=(kc == ET - 1))
                eng = CFG["evac_q"] if dst is qt else CFG["evac_k"]
                cut2 = CFG.get("evac_split", 0)
                if eng == "scalar":
                    nc.scalar.activation(
                        dst[:, t, :], ps[:], AF.Identity,
                        bias=b_sb[:, t:t + 1], scale=1.0)
                elif cut2:
                    nc.vector.tensor_scalar_add(
                        dst[:, t, 0:cut2], ps[:, 0:cut2],
                        b_sb[:, t:t + 1])
                    nc.scalar.activation(
                        dst[:, t, cut2:], ps[:, cut2:], AF.Identity,
                        bias=b_sb[:, t:t + 1], scale=1.0)
                else:
                    nc.vector.tensor_scalar_add(
                        dst[:, t, :], ps[:], b_sb[:, t:t + 1])

        # deferred last head pair of the previous batch: its P@V + norm
        # fill the batch-boundary pipeline bubble
        if env["pending_pv"] is not None:
            for hh in range(2):
                issue_o_head(env["pending_pv"], H // 2 - 1, hh)
            issue_norm(env["pending_pv"], H // 2 - 1)
            env["pending_pv"] = None

        # ---- v projection, n-major ([n, (h, d)]) + ones column
        vx = vx_pool.tile([P, NT, H, DH + 1], BF16)
        nc.gpsimd.memset(vx[:, :, :, DH:DH + 1], 1.0)
        for nt in range(NT):
            ps = ps_small.tile([P, E], FP32, tag="ps")
            for kc in range(ET):
                nc.tensor.matmul(
                    ps[:], xt[:, kc, nt * P:(nt + 1) * P],
                    wv_sb[:, kc, :],
                    start=(kc == 0), stop=(kc == ET - 1))
            nc.vector.tensor_add(
                vx[:, nt, :, 0:DH],
                ps.rearrange("p (h d) -> p h d", h=H),
                bv_sb.rearrange("p (h d) -> p h d", h=H))
        if b + 1 < BPC:
            env["next_x"] = load_x(b + 1)

        # ---- attention, head pairs (even head on PE rows 0-63, odd on
        # 64-127; scores land transposed [k, q])
        o_sb = o_pool.tile([P, NT, E], BF16)
        ot = ot_pool.tile([P, ET, N], BF16)
        adj_flat = adj_sb.rearrange("p c q -> p (c q)")
        st = (b, None, vx, o_sb, {}, ot)  # pts filled per pair

        def issue_scores(hp, ilv=None):
            t = hp
            pts = [pt_pool.tile([P, NT * N], BF16, tag="pt",
                                name=f"pt_{b}_{hp}_{i}")
                   for i in range(2)]
            for half in range(2):
                pss = [ps_big.tile([P, 2 * N], FP32, tag="scores",
                                   name=f"ss_{b}_{hp}_{half}_{i}")
                       for i in range(2)]
                for k2 in range(2):
                    kt = half * 2 + k2
                    for hh in range(2):
                        po = hh * DH
                        nc.tensor.matmul(
                            pss[hh][:, k2 * N:(k2 + 1) * N],
                            ktl[po:po + DH, t, kt * P:(kt + 1) * P],
                            qt[po:po + DH, t, :],
                            start=True, stop=True)
                lo = half * 2 * N
                fr = CFG["pool_mask_frac"]  # quarters of each chunk on Pool
                cut = lo + int((4 - fr) * N) // 2
                hi = (half + 1) * 2 * N
                for hh in range(2):
                    nc.scalar.activation(pts[hh][:, lo:hi], pss[hh][:],
                                         AF.Exp, scale=0.125)
                    nc.vector.tensor_tensor(
                        pts[hh][:, lo:cut], pts[hh][:, lo:cut],
                        adj_flat[:, lo:cut], op=MUL)
                    if fr:
                        nc.gpsimd.tensor_tensor(
                            pts[hh][:, cut:hi], pts[hh][:, cut:hi],
                            adj_flat[:, cut:hi], op=MUL)
                if ilv is not None:
                    issue_o_head(ilv, hp - 1, half)
                    if half == 1:
                        issue_norm(ilv, hp - 1)
            return pts

        def issue_final(args):
            bprev, o_prev, otprev = args
            for nt in range(NT):
                ps_f = ps_small.tile([P, E], FP32, tag="ps")
                for et in range(ET):
                    nc.tensor.matmul(
                        ps_f[:], otprev[:, et, nt * P:(nt + 1) * P],
                        wo_sb[:, et, :],
                        start=(et == 0), stop=False)
                nc.tensor.matmul(ps_f[:], ones1[:], bo1[:],
                                 start=False, stop=True)
                ob = out_pool.tile([P, E], FP32, tag="ob",
                                   name=f"ob_{bprev}_{nt}")
                nc.vector.tensor_copy(ob[:], ps_f[:])
                out_q.dma_start(
                    out_d.ap()[bprev, nt * P:(nt + 1) * P, :], ob[:])

        # interleave prev pair's o-stage between this pair's score chunks
        prev = None
        for hp in range(H // 2):
            cur = issue_scores(hp, ilv=(b, prev, vx, o_sb, st[4], ot)
                               if prev is not None else None)
            if hp == 2 and env["pending_final"] is not None:
                issue_final(env["pending_final"])
                env["pending_final"] = None
            prev = cur
        if CFG["defer_pv"]:
            env["pending_pv"] = (b, prev, vx, o_sb, st[4], ot)
        else:
            last = (b, prev, vx, o_sb, st[4], ot)
            for hh in range(2):
                issue_o_head(last, H // 2 - 1, hh)
            issue_norm(last, H // 2 - 1)
        env["pending_final"] = (b, o_sb, ot)

    # drain the final deferred pair, then the last batch's output stage
    if env["pending_pv"] is not None:
        for hh in range(2):
            issue_o_head(env["pending_pv"], H // 2 - 1, hh)
        issue_norm(env["pending_pv"], H // 2 - 1)
        env["pending_pv"] = None
    bprev, o_prev, otprev = env["pending_final"]
    for nt in range(NT):
        ps_f = ps_small.tile([P, E], FP32, tag="ps")
        for et in range(ET):
            nc.tensor.matmul(
                ps_f[:], otprev[:, et, nt * P:(nt + 1) * P],
                wo_sb[:, et, :],
                start=(et == 0), stop=False)
        nc.tensor.matmul(ps_f[:], ones1[:], bo1[:], start=False, stop=True)
        ob = out_pool.tile([P, E], FP32, tag="ob", name=f"ob_{bprev}_{nt}")
        nc.vector.tensor_copy(ob[:], ps_f[:])
        out_q.dma_start(out_d.ap()[bprev, nt * P:(nt + 1) * P, :], ob[:])


_NC_CACHE = {}


def get_nc(loop_iters=1):
    if loop_iters not in _NC_CACHE:
        _NC_CACHE[loop_iters] = build_nc(loop_iters)
    return _NC_CACHE[loop_iters]


def prep_inputs(x, adj, Wq, Wk, Wv, bq, bk, bv, Wo, bo):
    """Host-side layout prep -> per-core input maps."""
    import ml_dtypes
    x = np.asarray(x, dtype=np.float32)
    F8 = ml_dtypes.float8_e4m3
    shared = {
        "WvT": np.ascontiguousarray(np.asarray(Wv, np.float32).T.astype(ml_dtypes.bfloat16)),
        "WoT": np.ascontiguousarray(np.asarray(Wo, np.float32).T.astype(ml_dtypes.bfloat16)),
        "bqT": np.ascontiguousarray(np.asarray(bq, np.float32).reshape(ET, P).T),
        "bkT": np.ascontiguousarray(np.asarray(bk, np.float32).reshape(ET, P).T),
        "bvB": np.ascontiguousarray(
            np.broadcast_to(np.asarray(bv, np.float32), (P, E))),
        "boB": np.ascontiguousarray(
            np.broadcast_to(np.asarray(bo, np.float32), (P, E))),
        "adjT": np.ascontiguousarray(
            np.asarray(adj).T.astype(ml_dtypes.bfloat16)),
        "bo16": np.ascontiguousarray(
            np.asarray(bo, np.float32).reshape(1, E).astype(
                ml_dtypes.bfloat16)),
    }
    if CFG["qk_fp8"]:
        shared["Wq8"] = np.ascontiguousarray(
            np.asarray(Wq, np.float32).T.astype(F8))
        shared["Wk8"] = np.ascontiguousarray(
            np.asarray(Wk, np.float32).T.astype(F8))
    else:
        shared["Wq8"] = np.ascontiguousarray(
            np.asarray(Wq, np.float32).T.astype(ml_dtypes.bfloat16))
        shared["Wk8"] = np.ascontiguousarray(
            np.asarray(Wk, np.float32).T.astype(ml_dtypes.bfloat16))
    in_maps = []
    for c in range(N_CORES):
        xs = x[c * BPC:(c + 1) * BPC]  # [BPC, N, E]
        m = dict(shared)
        xsT = xs.transpose(0, 2, 1)
        m["xT"] = np.ascontiguousarray(xsT.astype(ml_dtypes.bfloat16))
        if CFG["qk_fp8"]:
            m["xT8"] = np.ascontiguousarray(xsT.astype(F8))
        in_maps.append(m)
    return in_maps


def kernel(**inputs):
    import os
    # this container lacks the axon NTFF hook; never attempt tracing
    os.environ.setdefault("BASS_NEVER_TRACE", "1")
    nc = get_nc()
    in_maps = prep_inputs(**inputs)
    res = bass_utils.run_bass_kernel_spmd(
        nc, in_maps, core_ids=list(range(N_CORES)))
    return np.concatenate([r["out"] for r in res.results], axis=0)


# ---------------------------------------------------------------------------
# Benchmarking helpers (not used by the grading path). Runs the kernel with
# inputs resident on device, with the whole per-core computation repeated
# R times inside the NEFF (tc.For_i); HW time per iteration is estimated as
# (T(R2) - T(R1)) / (R2 - R1) to cancel the fixed dispatch overhead.
def _make_sharded_fn(nc):
    import jax
    from jax.sharding import Mesh, PartitionSpec, NamedSharding
    from jax.experimental.shard_map import shard_map
    from concourse import bass2jax

    bass2jax.install_neuronx_cc_hook()
    pid = nc.partition_id_tensor
    in_names, out_names, out_avals = [], [], []
    for alloc in nc.m.functions[0].allocations:
        if not isinstance(alloc, mybir.MemoryLocationSet):
            continue
        name = alloc.memorylocations[0].name
        if alloc.kind == "ExternalInput":
            if pid is None or name != pid.name:
                in_names.append(name)
        elif alloc.kind == "ExternalOutput":
            out_names.append(name)
            out_avals.append(jax.core.ShapedArray(
                tuple(alloc.tensor_shape), mybir.dt.np(alloc.dtype)))
    all_in_names = in_names + out_names
    if pid is not None:
        all_in_names.append(pid.name)

    def _body(*args):
        operands = list(args)
        if pid is not None:
            operands.append(bass2jax.partition_id_tensor())
        return tuple(bass2jax._bass_exec_p.bind(
            *operands,
            out_avals=tuple(out_avals),
            in_names=tuple(all_in_names),
            out_names=tuple(out_names),
            lowering_input_output_aliases=(),
            sim_require_finite=True,
            sim_require_nnan=True,
            nc=nc,
        ))

    devices = jax.devices()[:N_CORES]
    mesh = Mesh(np.asarray(devices), ("core",))
    spec = PartitionSpec("core")
    nin = len(in_names) + len(out_names)
    fn = jax.jit(
        shard_map(_body, mesh=mesh, in_specs=(spec,) * nin,
                  out_specs=(spec,) * len(out_names), check_rep=False),
        keep_unused=True,
    )
    return fn, in_names, out_names, out_avals, mesh, spec


def _time_nc(nc, in_maps, n_rep):
    import time
    import jax
    from jax.sharding import NamedSharding

    fn, in_names, out_names, out_avals, mesh, spec = _make_sharded_fn(nc)
    sh = NamedSharding(mesh, spec)
    args = []
    for name in in_names:
        args.append(jax.device_put(
            np.concatenate([m[name] for m in in_maps], axis=0), sh))
    for av in out_avals:
        args.append(jax.device_put(
            np.zeros((N_CORES * av.shape[0],) + av.shape[1:], av.dtype), sh))
    out = fn(*args)
    jax.block_until_ready(out)
    ts = []
    for _ in range(n_rep):
        t0 = time.perf_counter()
        out = fn(*args)
        jax.block_until_ready(out)
        ts.append(time.perf_counter() - t0)
    return min(ts), out


def benchmark(inputs, r1=256, r2=1024, n_rep=10):
    """Interleaved two-point measurement: the ~80 ms axon dispatch overhead
    (and its drift) cancels in the difference; device time dominates both."""
    import time
    import jax
    from jax.sharding import NamedSharding

    in_maps = prep_inputs(**inputs)

    def setup(r):
        nc = get_nc(r)
        fn, in_names, out_names, out_avals, mesh, spec = _make_sharded_fn(nc)
        sh = NamedSharding(mesh, spec)
        args = []
        for name in in_names:
            args.append(jax.device_put(
                np.concatenate([m[name] for m in in_maps], axis=0), sh))
        for av in out_avals:
            args.append(jax.device_put(
                np.zeros((N_CORES * av.shape[0],) + av.shape[1:], av.dtype),
                sh))
        out = fn(*args)
        jax.block_until_ready(out)
        return fn, args

    f1, a1 = setup(r1)
    f2, a2 = setup(r2)
    t1s, t2s = [], []
    for _ in range(n_rep):
        t0 = time.perf_counter()
        jax.block_until_ready(f1(*a1))
        t1s.append(time.perf_counter() - t0)
        t0 = time.perf_counter()
        jax.block_until_ready(f2(*a2))
        t2s.append(time.perf_counter() - t0)
    return (min(t2s) - min(t1s)) * 1e9 / (r2 - r1)


# revision 39
# speedup vs baseline: 1.0167x; 1.0167x over previous
"""Masked multi-head attention (B=32, N=512, E=512, H=8) on 8 Trainium2 cores.

Sharding: data-parallel over batch (4 batches per core); weights and the
attention mask are replicated. All layout transforms (weight transposes,
x transpose, mask transforms, bias broadcast, bf16/fp8 casts) are host-side
numpy, so the device kernel is pure matmul/softmax work.

Per-core pipeline (per batch; scores/PV math bf16, Q/K proj fp8, psum fp32):
  qT = Wq8.T @ x8 (+bq)      fp8e4m3 DoubleRow matmuls (2 k-chunks/pass),
  kT = Wk8.T @ x8 (+bk)      e-major [e_out, n]; bias via ScalarE [P,1] add
  v  = xT.T @ WvT (+bv)      bf16, n-major, ones column appended per head so
                             the softmax denominator falls out of the P@V mm
  per head pair:
    psum = kT_h.T @ qT_h     scores transposed [k, q], two banks per chunk
    P    = exp(psum / 8)     one ACTIVATE per 2 banks, scale fused, bf16 out
    P   *= adj.T             VectorE mask multiply (scores are small, so no
                             max-subtraction is needed)
  P@V: per (pair, qi-pair) ONE psum bank holds 4 groups [qi&1, hh] of
    [q, v_h | 1] at 128-col offsets (col 64 = denominator); normalization is
    a strided reciprocal + one broadcast 4D multiply per bank (24 DVE
    ops/batch vs 256); the last pair's P@V+norm defer into the next batch
    to fill the boundary bubble.
  oT = DMA-transpose(o)      per-pair, right after each norm (spreads HWDGE)
  out = oT.T @ WoT (+bo via a K=1 ones-row matmul); psum evacuated by DVE;
                             interleaved into the NEXT batch's attention
  Mask multiplies split ~3:1 between DVE and GpSimd; q/k evacuations are
  DVE TensorScalarPtr ops so ScalarE runs exp-only (no act-table swaps).
"""

import numpy as np

import concourse.bass as bass
import concourse.tile as tile
from concourse import bacc, mybir
import concourse.bass_utils as bass_utils

N_CORES = 8
B, N, E, H = 32, 512, 512, 8
DH = E // H  # 64
BPC = B // N_CORES  # batches per core
P = 128
NT = N // P  # 4 tiles along sequence
ET = E // P  # 4 tiles along embedding
FP32 = mybir.dt.float32
BF16 = mybir.dt.bfloat16
FP8 = mybir.dt.float8e4
DR = mybir.MatmulPerfMode.DoubleRow
AF = mybir.ActivationFunctionType


# tunable knobs
CFG = {
    "xt": 2, "qt": 2, "kt": 2, "vx": 2, "pt": 6, "ot": 3, "out": 4,
    "small": 8, "scores": 2, "ps": 2, "pso": 2,
    "qk_fp8": True, "dma_q": "sync", "trans_q": "sync", "out_q": "sync",
    "pack_norm": True, "defer_pv": True, "pool_mask_frac": 1, "evac_q": "vector", "evac_k": "vector", "hoist_s0": True,
}


def build_nc(loop_iters=1):
    nc = bacc.Bacc("TRN2", target_bir_lowering=False, debug=False,
                   num_devices=N_CORES)

    xT_d = nc.dram_tensor("xT", [BPC, E, N], BF16, kind="ExternalInput")
    x8_d = (nc.dram_tensor("xT8", [BPC, E, N], FP8, kind="ExternalInput")
            if CFG["qk_fp8"] else None)
    qdt = FP8 if CFG["qk_fp8"] else BF16
    wq_d = nc.dram_tensor("Wq8", [E, E], qdt, kind="ExternalInput")
    wk_d = nc.dram_tensor("Wk8", [E, E], qdt, kind="ExternalInput")
    wv_d = nc.dram_tensor("WvT", [E, E], BF16, kind="ExternalInput")
    wo_d = nc.dram_tensor("WoT", [E, E], BF16, kind="ExternalInput")
    bq_d = nc.dram_tensor("bqT", [P, ET], FP32, kind="ExternalInput")
    bk_d = nc.dram_tensor("bkT", [P, ET], FP32, kind="ExternalInput")
    bv_d = nc.dram_tensor("bvB", [P, E], FP32, kind="ExternalInput")
    bo_d = nc.dram_tensor("boB", [P, E], FP32, kind="ExternalInput")
    adj_d = nc.dram_tensor("adjT", [N, N], BF16, kind="ExternalInput")
    bo16_d = nc.dram_tensor("bo16", [1, E], BF16, kind="ExternalInput")
    out_d = nc.dram_tensor("out", [BPC, N, E], FP32, kind="ExternalOutput")

    with tile.TileContext(nc) as tc:
        with (
            tc.tile_pool(name="persist", bufs=1) as persist,
            tc.tile_pool(name="xt", bufs=CFG["xt"]) as xt_pool,
            tc.tile_pool(name="x8", bufs=CFG["xt"]) as x8_pool,
            tc.tile_pool(name="qt", bufs=CFG["qt"]) as qt_pool,
            tc.tile_pool(name="kt", bufs=CFG["kt"]) as kt_pool,
            tc.tile_pool(name="vx", bufs=CFG["vx"]) as vx_pool,
            tc.tile_pool(name="pt", bufs=CFG["pt"]) as pt_pool,
            tc.tile_pool(name="osb", bufs=2) as o_pool,
            tc.tile_pool(name="otsb", bufs=CFG["ot"]) as ot_pool,
            tc.tile_pool(name="outsb", bufs=CFG["out"]) as out_pool,
            tc.tile_pool(name="small", bufs=CFG["small"]) as small_pool,
            tc.tile_pool(name="ps_big", bufs=CFG["scores"], space="PSUM") as ps_big,
            tc.tile_pool(name="ps_small", bufs=CFG["ps"], space="PSUM") as ps_small,
            tc.tile_pool(name="ps_o", bufs=CFG["pso"], space="PSUM") as ps_o_pool,
        ):
            # ---- persistent tensors (replicated weights / mask / biases)
            wq_sb = persist.tile([P, ET, E], qdt)
            nc.sync.dma_start(wq_sb[:], wq_d.ap().rearrange("(c p) e -> p c e", p=P))
            bq_sb = persist.tile([P, ET], FP32)
            nc.sync.dma_start(bq_sb[:], bq_d.ap())
            wk_sb = persist.tile([P, ET, E], qdt)
            nc.sync.dma_start(wk_sb[:], wk_d.ap().rearrange("(c p) e -> p c e", p=P))
            bk_sb = persist.tile([P, ET], FP32)
            nc.sync.dma_start(bk_sb[:], bk_d.ap())
            wv_sb = persist.tile([P, ET, E], BF16)
            bv_sb = persist.tile([P, E], FP32)
            adj_sb = persist.tile([P, NT, N], BF16)
            wo_sb = persist.tile([P, ET, E], BF16)
            bo_sb = persist.tile([P, E], FP32)
            ones1 = persist.tile([1, P], BF16)
            nc.vector.memset(ones1[:], 1.0)
            bo1 = persist.tile([1, E], BF16)

            def load_persist2(loop_iters=loop_iters):
                nc.scalar.dma_start(
                    adj_sb[:],
                    adj_d.ap().rearrange("(c p) q -> p c q", p=P))
                nc.scalar.dma_start(
                    wv_sb[:], wv_d.ap().rearrange("(c p) e -> p c e", p=P))
                nc.scalar.dma_start(bv_sb[:], bv_d.ap())
                nc.scalar.dma_start(
                    wo_sb[:], wo_d.ap().rearrange("(c p) e -> p c e", p=P))
                nc.scalar.dma_start(bo_sb[:], bo_d.ap())
                nc.scalar.dma_start(bo1[:], bo16_d.ap())

            import contextlib
            if loop_iters > 1:
                load_persist2()
            loop_cm = (tc.For_i(0, loop_iters, 1) if loop_iters > 1
                       else contextlib.nullcontext())
            with loop_cm:
                body(nc, tc, locals())

    nc.compile()
    return nc


def body(nc, tc, env):
    (xT_d, x8_d, out_d, wq_sb, wk_sb, wv_sb, wo_sb, adj_sb, bq_sb, bk_sb,
     bv_sb, bo_sb, ones1, bo1) = (env[k] for k in (
         "xT_d", "x8_d", "out_d", "wq_sb", "wk_sb", "wv_sb", "wo_sb",
         "adj_sb", "bq_sb", "bk_sb", "bv_sb", "bo_sb", "ones1", "bo1"))

    (xt_pool, x8_pool, qt_pool, kt_pool, vx_pool, pt_pool, o_pool, ot_pool,
     out_pool, small_pool, ps_big, ps_small, ps_o_pool) = (env[k] for k in (
         "xt_pool", "x8_pool", "qt_pool", "kt_pool", "vx_pool", "pt_pool",
         "o_pool", "ot_pool", "out_pool", "small_pool", "ps_big", "ps_small",
         "ps_o_pool"))
    MUL = mybir.AluOpType.mult
    env["pending_final"] = None
    env["pending_pv"] = None
    dma_q = getattr(nc, CFG["dma_q"])
    trans_q = getattr(nc, CFG["trans_q"])
    out_q = getattr(nc, CFG["out_q"])

    def issue_o_head(st, hp, hh):
        # P@V for head h = 2*hp + hh; psum bank per (hp, qi-pair) holds
        # four 65-col groups [qi&1, hh] at 128-col offsets
        bb, pts, vx, o_sb, pso_live, ot = st
        h = 2 * hp + hh
        for qi in range(NT):
            qi2 = qi // 2
            if hh == 0 and qi % 2 == 0:
                pso_live[qi2] = env["ps_o_pool"].tile(
                    [P, 4, DH + 1], FP32, tag="pso",
                    padded_shape=[P, 4, P], name=f"pso_{bb}_{hp}_{qi2}")
            ps_o = pso_live[qi2]
            j = (qi % 2) * 2 + hh
            for kt in range(NT):
                nc.tensor.matmul(
                    ps_o[:, j, 0:DH + 1],
                    pts[hh][:, kt * N + qi * P:kt * N + qi * P + P],
                    vx[:, kt, h, :],
                    start=(kt == 0), stop=(kt == NT - 1))

    def issue_norm(st, hp):
        # o[q, h*64:(h+1)*64] = num / den for the pair's 4 (qi&1, hh)
        # groups per bank: strided reciprocal + broadcast multiply
        bb, pts, vx, o_sb, pso_live, ot = st
        for qi2 in range(2):
            ps_o = pso_live.pop(qi2)
            rc = env["small_pool"].tile([P, 4, 1], FP32, tag="rc",
                                        name=f"rc_{bb}_{hp}_{qi2}")
            nc.vector.reciprocal(rc[:], ps_o[:, :, DH:DH + 1])
            nc.vector.tensor_tensor(
                o_sb[:, 2 * qi2:2 * qi2 + 2,
                     hp * 2 * DH:(hp + 1) * 2 * DH].rearrange(
                    "p q (c d) -> p q c d", d=DH),
                ps_o[:, :, 0:DH].rearrange("p (a c) d -> p a c d", c=2),
                rc.rearrange("p (a c) d -> p a c d", c=2).broadcast_to(
                    [P, 2, 2, DH]),
                op=MUL)
        for nt in range(NT):
            trans_q.dma_start_transpose(
                ot[:, hp, nt * P:(nt + 1) * P],
                o_sb[:, nt, hp * P:(hp + 1) * P])

    def load_x(b):
        x8 = None
        if CFG["qk_fp8"]:
            x8 = x8_pool.tile([P, ET, N], FP8, name=f"x8_{b}")
            dma_q.dma_start(
                x8[:], x8_d.ap()[b].rearrange("(c p) n -> p c n", p=P))
        xt = xt_pool.tile([P, ET, N], BF16, name=f"xt_{b}")
        dma_q.dma_start(xt[:], xT_d.ap()[b].rearrange("(c p) n -> p c n", p=P))
        return xt, x8

    env["next_x"] = load_x(0)
    if env["loop_iters"] == 1:
        env["load_persist2"]()
    for b in range(BPC):
        xt, x8 = env["next_x"]

        # ---- q/k projections, e-major output (qT[e_out, n])
        qt = qt_pool.tile([P, ET, N], BF16)
        ktl = kt_pool.tile([P, ET, N], BF16)
        for t in range(ET):
            for w_sb, b_sb, dst in ((wq_sb, bq_sb, qt), (wk_sb, bk_sb, ktl)):
                ps = ps_small.tile([P, N], FP32, tag="ps")
                if CFG["qk_fp8"]:
                    for c in range(ET // 2):
                        nc.tensor.matmul(
                            ps[:],
                            w_sb[:, 2 * c:2 * c + 2, t * P:(t + 1) * P],
                            x8[:, 2 * c:2 * c + 2, :],
                            start=(c == 0), stop=(c == ET // 2 - 1),
                            perf_mode=DR)
                else:
                    for kc in range(ET):
                        nc.tensor.matmul(
                            ps[:], w_sb[:, kc, t * P:(t + 1) * P],
                            xt[:, kc, :],
                            start=(kc == 0), stop=(kc == ET - 1))
                eng = CFG["evac_q"] if dst is qt else CFG["evac_k"]
                cut2 = CFG.get("evac_split", 0)
                if eng == "scalar":
                    nc.scalar.activation(
                        dst[:, t, :], ps[:], AF.Identity,
                        bias=b_sb[:, t:t + 1], scale=1.0)
                elif cut2:
                    nc.vector.tensor_scalar_add(
                        dst[:, t, 0:cut2], ps[:, 0:cut2],
                        b_sb[:, t:t + 1])
                    nc.scalar.activation(
                        dst[:, t, cut2:], ps[:, cut2:], AF.Identity,
                        bias=b_sb[:, t:t + 1], scale=1.0)
                else:
                    nc.vector.tensor_scalar_add(
                        dst[:, t, :], ps[:], b_sb[:, t:t + 1])

        # deferred last head pair of the previous batch: its P@V + norm
        # fill the batch-boundary pipeline bubble
        if env["pending_pv"] is not None:
            for hh in range(2):
                issue_o_head(env["pending_pv"], H // 2 - 1, hh)
            issue_norm(env["pending_pv"], H // 2 - 1)
            env["pending_pv"] = None

        # ---- v projection, n-major ([n, (h, d)]) + ones column
        hoist = CFG.get("hoist_s0", False)
        vx = vx_pool.tile([P, NT, H, DH + 1], BF16)
        nc.gpsimd.memset(vx[:, :, :, DH:DH + 1], 1.0)

        def issue_vproj():
            for nt in range(NT):
                ps = ps_small.tile([P, E], FP32, tag="ps")
                for kc in range(ET):
                    nc.tensor.matmul(
                        ps[:], xt[:, kc, nt * P:(nt + 1) * P],
                        wv_sb[:, kc, :],
                        start=(kc == 0), stop=(kc == ET - 1))
                nc.vector.tensor_add(
                    vx[:, nt, :, 0:DH],
                    ps.rearrange("p (h d) -> p h d", h=H),
                    bv_sb.rearrange("p (h d) -> p h d", h=H))
            if b + 1 < BPC:
                env["next_x"] = load_x(b + 1)

        if not hoist:
            issue_vproj()

        # ---- attention, head pairs (even head on PE rows 0-63, odd on
        # 64-127; scores land transposed [k, q])
        o_sb = o_pool.tile([P, NT, E], BF16)
        ot = ot_pool.tile([P, ET, N], BF16)
        adj_flat = adj_sb.rearrange("p c q -> p (c q)")
        st = (b, None, vx, o_sb, {}, ot)  # pts filled per pair

        def issue_scores(hp, ilv=None):
            t = hp
            pts = [pt_pool.tile([P, NT * N], BF16, tag="pt",
                                name=f"pt_{b}_{hp}_{i}")
                   for i in range(2)]
            for half in range(2):
                pss = [ps_big.tile([P, 2 * N], FP32, tag="scores",
                                   name=f"ss_{b}_{hp}_{half}_{i}")
                       for i in range(2)]
                for k2 in range(2):
                    kt = half * 2 + k2
                    for hh in range(2):
                        po = hh * DH
                        nc.tensor.matmul(
                            pss[hh][:, k2 * N:(k2 + 1) * N],
                            ktl[po:po + DH, t, kt * P:(kt + 1) * P],
                            qt[po:po + DH, t, :],
                            start=True, stop=True)
                lo = half * 2 * N
                fr = CFG["pool_mask_frac"]  # quarters of each chunk on Pool
                cut = lo + int((4 - fr) * N) // 2
                hi = (half + 1) * 2 * N
                for hh in range(2):
                    nc.scalar.activation(pts[hh][:, lo:hi], pss[hh][:],
                                         AF.Exp, scale=0.125)
                    nc.vector.tensor_tensor(
                        pts[hh][:, lo:cut], pts[hh][:, lo:cut],
                        adj_flat[:, lo:cut], op=MUL)
                    if fr:
                        nc.gpsimd.tensor_tensor(
                            pts[hh][:, cut:hi], pts[hh][:, cut:hi],
                            adj_flat[:, cut:hi], op=MUL)
                if ilv is not None:
                    issue_o_head(ilv, hp - 1, half)
                    if half == 1:
                        issue_norm(ilv, hp - 1)
            return pts

        def issue_final(args):
            bprev, o_prev, otprev = args
            for nt in range(NT):
                ps_f = ps_small.tile([P, E], FP32, tag="ps")
                for et in range(ET):
                    nc.tensor.matmul(
                        ps_f[:], otprev[:, et, nt * P:(nt + 1) * P],
                        wo_sb[:, et, :],
                        start=(et == 0), stop=False)
                nc.tensor.matmul(ps_f[:], ones1[:], bo1[:],
                                 start=False, stop=True)
                ob = out_pool.tile([P, E], FP32, tag="ob",
                                   name=f"ob_{bprev}_{nt}")
                nc.vector.tensor_copy(ob[:], ps_f[:])
                out_q.dma_start(
                    out_d.ap()[bprev, nt * P:(nt + 1) * P, :], ob[:])

        # interleave prev pair's o-stage between this pair's score chunks
        prev = None
        for hp in range(H // 2):
            cur = issue_scores(hp, ilv=(b, prev, vx, o_sb, st[4], ot)
                               if prev is not None else None)
            if hp == 0 and hoist:
                issue_vproj()
            if hp == 2 and env["pending_final"] is not None:
                issue_final(env["pending_final"])
                env["pending_final"] = None
            prev = cur
        if CFG["defer_pv"]:
            env["pending_pv"] = (b, prev, vx, o_sb, st[4], ot)
        else:
            last = (b, prev, vx, o_sb, st[4], ot)
            for hh in range(2):
                issue_o_head(last, H // 2 - 1, hh)
            issue_norm(last, H // 2 - 1)
        env["pending_final"] = (b, o_sb, ot)

    # drain the final deferred pair, then the last batch's output stage
    if env["pending_pv"] is not None:
        for hh in range(2):
            issue_o_head(env["pending_pv"], H // 2 - 1, hh)
        issue_norm(env["pending_pv"], H // 2 - 1)
        env["pending_pv"] = None
    bprev, o_prev, otprev = env["pending_final"]
    for nt in range(NT):
        ps_f = ps_small.tile([P, E], FP32, tag="ps")
        for et in range(ET):
            nc.tensor.matmul(
                ps_f[:], otprev[:, et, nt * P:(nt + 1) * P],
                wo_sb[:, et, :],
                start=(et == 0), stop=False)
        nc.tensor.matmul(ps_f[:], ones1[:], bo1[:], start=False, stop=True)
        ob = out_pool.tile([P, E], FP32, tag="ob", name=f"ob_{bprev}_{nt}")
        nc.vector.tensor_copy(ob[:], ps_f[:])
        out_q.dma_start(out_d.ap()[bprev, nt * P:(nt + 1) * P, :], ob[:])


_NC_CACHE = {}


def get_nc(loop_iters=1):
    if loop_iters not in _NC_CACHE:
        _NC_CACHE[loop_iters] = build_nc(loop_iters)
    return _NC_CACHE[loop_iters]


def prep_inputs(x, adj, Wq, Wk, Wv, bq, bk, bv, Wo, bo):
    """Host-side layout prep -> per-core input maps."""
    import ml_dtypes
    x = np.asarray(x, dtype=np.float32)
    F8 = ml_dtypes.float8_e4m3
    shared = {
        "WvT": np.ascontiguousarray(np.asarray(Wv, np.float32).T.astype(ml_dtypes.bfloat16)),
        "WoT": np.ascontiguousarray(np.asarray(Wo, np.float32).T.astype(ml_dtypes.bfloat16)),
        "bqT": np.ascontiguousarray(np.asarray(bq, np.float32).reshape(ET, P).T),
        "bkT": np.ascontiguousarray(np.asarray(bk, np.float32).reshape(ET, P).T),
        "bvB": np.ascontiguousarray(
            np.broadcast_to(np.asarray(bv, np.float32), (P, E))),
        "boB": np.ascontiguousarray(
            np.broadcast_to(np.asarray(bo, np.float32), (P, E))),
        "adjT": np.ascontiguousarray(
            np.asarray(adj).T.astype(ml_dtypes.bfloat16)),
        "bo16": np.ascontiguousarray(
            np.asarray(bo, np.float32).reshape(1, E).astype(
                ml_dtypes.bfloat16)),
    }
    if CFG["qk_fp8"]:
        shared["Wq8"] = np.ascontiguousarray(
            np.asarray(Wq, np.float32).T.astype(F8))
        shared["Wk8"] = np.ascontiguousarray(
            np.asarray(Wk, np.float32).T.astype(F8))
    else:
        shared["Wq8"] = np.ascontiguousarray(
            np.asarray(Wq, np.float32).T.astype(ml_dtypes.bfloat16))
        shared["Wk8"] = np.ascontiguousarray(
            np.asarray(Wk, np.float32).T.astype(ml_dtypes.bfloat16))
    in_maps = []
    for c in range(N_CORES):
        xs = x[c * BPC:(c + 1) * BPC]  # [BPC, N, E]
        m = dict(shared)
        xsT = xs.transpose(0, 2, 1)
        m["xT"] = np.ascontiguousarray(xsT.astype(ml_dtypes.bfloat16))
        if CFG["qk_fp8"]:
            m["xT8"] = np.ascontiguousarray(xsT.astype(F8))
        in_maps.append(m)
    return in_maps


def kernel(**inputs):
    import os
    # this container lacks the axon NTFF hook; never attempt tracing
    os.environ.setdefault("BASS_NEVER_TRACE", "1")
    nc = get_nc()
    in_maps = prep_inputs(**inputs)
    res = bass_utils.run_bass_kernel_spmd(
        nc, in_maps, core_ids=list(range(N_CORES)))
    return np.concatenate([r["out"] for r in res.results], axis=0)


# ---------------------------------------------------------------------------
# Benchmarking helpers (not used by the grading path). Runs the kernel with
# inputs resident on device, with the whole per-core computation repeated
# R times inside the NEFF (tc.For_i); HW time per iteration is estimated as
# (T(R2) - T(R1)) / (R2 - R1) to cancel the fixed dispatch overhead.
def _make_sharded_fn(nc):
    import jax
    from jax.sharding import Mesh, PartitionSpec, NamedSharding
    from jax.experimental.shard_map import shard_map
    from concourse import bass2jax

    bass2jax.install_neuronx_cc_hook()
    pid = nc.partition_id_tensor
    in_names, out_names, out_avals = [], [], []
    for alloc in nc.m.functions[0].allocations:
        if not isinstance(alloc, mybir.MemoryLocationSet):
            continue
        name = alloc.memorylocations[0].name
        if alloc.kind == "ExternalInput":
            if pid is None or name != pid.name:
                in_names.append(name)
        elif alloc.kind == "ExternalOutput":
            out_names.append(name)
            out_avals.append(jax.core.ShapedArray(
                tuple(alloc.tensor_shape), mybir.dt.np(alloc.dtype)))
    all_in_names = in_names + out_names
    if pid is not None:
        all_in_names.append(pid.name)

    def _body(*args):
        operands = list(args)
        if pid is not None:
            operands.append(bass2jax.partition_id_tensor())
        return tuple(bass2jax._bass_exec_p.bind(
            *operands,
            out_avals=tuple(out_avals),
            in_names=tuple(all_in_names),
            out_names=tuple(out_names),
            lowering_input_output_aliases=(),
            sim_require_finite=True,
            sim_require_nnan=True,
            nc=nc,
        ))

    devices = jax.devices()[:N_CORES]
    mesh = Mesh(np.asarray(devices), ("core",))
    spec = PartitionSpec("core")
    nin = len(in_names) + len(out_names)
    fn = jax.jit(
        shard_map(_body, mesh=mesh, in_specs=(spec,) * nin,
                  out_specs=(spec,) * len(out_names), check_rep=False),
        keep_unused=True,
    )
    return fn, in_names, out_names, out_avals, mesh, spec


def _time_nc(nc, in_maps, n_rep):
    import time
    import jax
    from jax.sharding import NamedSharding

    fn, in_names, out_names, out_avals, mesh, spec = _make_sharded_fn(nc)
    sh = NamedSharding(mesh, spec)
    args = []
    for name in in_names:
        args.append(jax.device_put(
            np.concatenate([m[name] for m in in_maps], axis=0), sh))
    for av in out_avals:
        args.append(jax.device_put(
            np.zeros((N_CORES * av.shape[0],) + av.shape[1:], av.dtype), sh))
    out = fn(*args)
    jax.block_until_ready(out)
    ts = []
    for _ in range(n_rep):
        t0 = time.perf_counter()
        out = fn(*args)
        jax.block_until_ready(out)
        ts.append(time.perf_counter() - t0)
    return min(ts), out


def benchmark(inputs, r1=256, r2=1024, n_rep=10):
    """Interleaved two-point measurement: the ~80 ms axon dispatch overhead
    (and its drift) cancels in the difference; device time dominates both."""
    import time
    import jax
    from jax.sharding import NamedSharding

    in_maps = prep_inputs(**inputs)

    def setup(r):
        nc = get_nc(r)
        fn, in_names, out_names, out_avals, mesh, spec = _make_sharded_fn(nc)
        sh = NamedSharding(mesh, spec)
        args = []
        for name in in_names:
            args.append(jax.device_put(
                np.concatenate([m[name] for m in in_maps], axis=0), sh))
        for av in out_avals:
            args.append(jax.device_put(
                np.zeros((N_CORES * av.shape[0],) + av.shape[1:], av.dtype),
                sh))
        out = fn(*args)
        jax.block_until_ready(out)
        return fn, args

    f1, a1 = setup(r1)
    f2, a2 = setup(r2)
    t1s, t2s = [], []
    for _ in range(n_rep):
        t0 = time.perf_counter()
        jax.block_until_ready(f1(*a1))
        t1s.append(time.perf_counter() - t0)
        t0 = time.perf_counter()
        jax.block_until_ready(f2(*a2))
        t2s.append(time.perf_counter() - t0)
    return (min(t2s) - min(t1s)) * 1e9 / (r2 - r1)


# revision 40
# speedup vs baseline: 1.1167x; 1.0983x over previous
"""Masked multi-head attention (B=32, N=512, E=512, H=8) on 8 Trainium2 cores.

Sharding: data-parallel over batch (4 batches per core); weights and the
attention mask are replicated. All layout transforms (weight transposes,
x transpose, mask transforms, bias broadcast, bf16/fp8 casts) are host-side
numpy, so the device kernel is pure matmul/softmax work.

Per-core pipeline (per batch; scores/PV math bf16, Q/K proj fp8, psum fp32):
  qT = Wq8.T @ x8 (+bq)      fp8e4m3 DoubleRow matmuls (2 k-chunks/pass),
  kT = Wk8.T @ x8 (+bk)      e-major [e_out, n]; bias via ScalarE [P,1] add
  v  = xT.T @ WvT (+bv)      bf16, n-major, ones column appended per head so
                             the softmax denominator falls out of the P@V mm
  per head pair:
    psum = kT_h.T @ qT_h     scores transposed [k, q], two banks per chunk
    P    = exp(psum / 8)     one ACTIVATE per 2 banks, scale fused, bf16 out
    P   *= adj.T             VectorE mask multiply (scores are small, so no
                             max-subtraction is needed)
  P@V: per (pair, qi-pair) ONE psum bank holds 4 groups [qi&1, hh] of
    [q, v_h | 1] at 128-col offsets (col 64 = denominator); normalization is
    a strided reciprocal + one broadcast 4D multiply per bank (24 DVE
    ops/batch vs 256); the last pair's P@V+norm defer into the next batch
    to fill the boundary bubble.
  oT = DMA-transpose(o)      per-pair, right after each norm (spreads HWDGE)
  out = oT.T @ WoT (+bo via a K=1 ones-row matmul); psum evacuated by DVE;
                             interleaved into the NEXT batch's attention
  Mask multiplies split ~3:1 between DVE and GpSimd; q/k evacuations are
  DVE TensorScalarPtr ops so ScalarE runs exp-only (no act-table swaps).
"""

import numpy as np

import concourse.bass as bass
import concourse.tile as tile
from concourse import bacc, mybir
import concourse.bass_utils as bass_utils

N_CORES = 8
B, N, E, H = 32, 512, 512, 8
DH = E // H  # 64
BPC = B // N_CORES  # batches per core
P = 128
NT = N // P  # 4 tiles along sequence
ET = E // P  # 4 tiles along embedding
FP32 = mybir.dt.float32
BF16 = mybir.dt.bfloat16
FP8 = mybir.dt.float8e4
DR = mybir.MatmulPerfMode.DoubleRow
AF = mybir.ActivationFunctionType


# tunable knobs
CFG = {
    "xt": 2, "qt": 2, "kt": 2, "vx": 2, "pt": 6, "ot": 3, "out": 4,
    "small": 8, "scores": 2, "ps": 2, "pso": 2,
    "qk_fp8": True, "dma_q": "sync", "trans_q": "sync", "out_q": "sync",
    "pack_norm": True, "defer_pv": True, "pool_mask_frac": 1, "evac_q": "vector", "evac_k": "vector", "hoist_s0": True,
}


def build_nc(loop_iters=1):
    nc = bacc.Bacc("TRN2", target_bir_lowering=False, debug=False,
                   num_devices=N_CORES)

    xT_d = nc.dram_tensor("xT", [BPC, E, N], BF16, kind="ExternalInput")
    x8_d = (nc.dram_tensor("xT8", [BPC, E, N], FP8, kind="ExternalInput")
            if CFG["qk_fp8"] else None)
    qdt = FP8 if CFG["qk_fp8"] else BF16
    wq_d = nc.dram_tensor("Wq8", [E, E], qdt, kind="ExternalInput")
    wk_d = nc.dram_tensor("Wk8", [E, E], qdt, kind="ExternalInput")
    wv_d = nc.dram_tensor("WvT", [E, E], BF16, kind="ExternalInput")
    wo_d = nc.dram_tensor("WoT", [E, E], BF16, kind="ExternalInput")
    bq_d = nc.dram_tensor("bqT", [P, ET], FP32, kind="ExternalInput")
    bk_d = nc.dram_tensor("bkT", [P, ET], FP32, kind="ExternalInput")
    bv_d = nc.dram_tensor("bvB", [P, E], FP32, kind="ExternalInput")
    bo_d = nc.dram_tensor("boB", [P, E], FP32, kind="ExternalInput")
    adj_d = nc.dram_tensor("adjT", [N, N], BF16, kind="ExternalInput")
    bo16_d = nc.dram_tensor("bo16", [1, E], BF16, kind="ExternalInput")
    out_d = nc.dram_tensor("out", [BPC, N, E], FP32, kind="ExternalOutput")

    with tile.TileContext(nc) as tc:
        with (
            tc.tile_pool(name="persist", bufs=1) as persist,
            tc.tile_pool(name="xt", bufs=CFG["xt"]) as xt_pool,
            tc.tile_pool(name="x8", bufs=CFG["xt"]) as x8_pool,
            tc.tile_pool(name="qt", bufs=CFG["qt"]) as qt_pool,
            tc.tile_pool(name="kt", bufs=CFG["kt"]) as kt_pool,
            tc.tile_pool(name="vx", bufs=CFG["vx"]) as vx_pool,
            tc.tile_pool(name="pt", bufs=CFG["pt"]) as pt_pool,
            tc.tile_pool(name="osb", bufs=CFG.get("osb", 2)) as o_pool,
            tc.tile_pool(name="otsb", bufs=CFG["ot"]) as ot_pool,
            tc.tile_pool(name="outsb", bufs=CFG["out"]) as out_pool,
            tc.tile_pool(name="small", bufs=CFG["small"]) as small_pool,
            tc.tile_pool(name="ps_big", bufs=CFG["scores"], space="PSUM") as ps_big,
            tc.tile_pool(name="ps_small", bufs=CFG["ps"], space="PSUM") as ps_small,
            tc.tile_pool(name="ps_o", bufs=CFG["pso"], space="PSUM") as ps_o_pool,
        ):
            # ---- persistent tensors (replicated weights / mask / biases)
            wq_sb = persist.tile([P, ET, E], qdt)
            nc.sync.dma_start(wq_sb[:], wq_d.ap().rearrange("(c p) e -> p c e", p=P))
            bq_sb = persist.tile([P, ET], FP32)
            nc.sync.dma_start(bq_sb[:], bq_d.ap())
            wk_sb = persist.tile([P, ET, E], qdt)
            nc.sync.dma_start(wk_sb[:], wk_d.ap().rearrange("(c p) e -> p c e", p=P))
            bk_sb = persist.tile([P, ET], FP32)
            nc.sync.dma_start(bk_sb[:], bk_d.ap())
            wv_sb = persist.tile([P, ET, E], BF16)
            bv_sb = persist.tile([P, E], FP32)
            adj_sb = persist.tile([P, NT, N], BF16)
            wo_sb = persist.tile([P, ET, E], BF16)
            bo_sb = persist.tile([P, E], FP32)
            ones1 = persist.tile([1, P], BF16)
            nc.vector.memset(ones1[:], 1.0)
            bo1 = persist.tile([1, E], BF16)

            def load_persist2(loop_iters=loop_iters):
                nc.scalar.dma_start(
                    adj_sb[:],
                    adj_d.ap().rearrange("(c p) q -> p c q", p=P))
                nc.scalar.dma_start(
                    wv_sb[:], wv_d.ap().rearrange("(c p) e -> p c e", p=P))
                nc.scalar.dma_start(bv_sb[:], bv_d.ap())
                nc.scalar.dma_start(
                    wo_sb[:], wo_d.ap().rearrange("(c p) e -> p c e", p=P))
                nc.scalar.dma_start(bo_sb[:], bo_d.ap())
                nc.scalar.dma_start(bo1[:], bo16_d.ap())

            import contextlib
            if loop_iters > 1:
                load_persist2()
            loop_cm = (tc.For_i(0, loop_iters, 1) if loop_iters > 1
                       else contextlib.nullcontext())
            with loop_cm:
                body(nc, tc, locals())

    nc.compile()
    return nc


def body(nc, tc, env):
    (xT_d, x8_d, out_d, wq_sb, wk_sb, wv_sb, wo_sb, adj_sb, bq_sb, bk_sb,
     bv_sb, bo_sb, ones1, bo1) = (env[k] for k in (
         "xT_d", "x8_d", "out_d", "wq_sb", "wk_sb", "wv_sb", "wo_sb",
         "adj_sb", "bq_sb", "bk_sb", "bv_sb", "bo_sb", "ones1", "bo1"))

    (xt_pool, x8_pool, qt_pool, kt_pool, vx_pool, pt_pool, o_pool, ot_pool,
     out_pool, small_pool, ps_big, ps_small, ps_o_pool) = (env[k] for k in (
         "xt_pool", "x8_pool", "qt_pool", "kt_pool", "vx_pool", "pt_pool",
         "o_pool", "ot_pool", "out_pool", "small_pool", "ps_big", "ps_small",
         "ps_o_pool"))
    MUL = mybir.AluOpType.mult
    env["pending_final"] = None
    env["pending_pv"] = None
    dma_q = getattr(nc, CFG["dma_q"])
    trans_q = getattr(nc, CFG["trans_q"])
    out_q = getattr(nc, CFG["out_q"])

    def issue_o_head(st, hp, hh):
        # P@V for head h = 2*hp + hh; psum bank per (hp, qi-pair) holds
        # four 65-col groups [qi&1, hh] at 128-col offsets
        bb, pts, vx, o_sb, pso_live, ot = st
        h = 2 * hp + hh
        for qi in range(NT):
            qi2 = qi // 2
            if hh == 0 and qi % 2 == 0:
                pso_live[qi2] = env["ps_o_pool"].tile(
                    [P, 4, DH + 1], FP32, tag="pso",
                    padded_shape=[P, 4, P], name=f"pso_{bb}_{hp}_{qi2}")
            ps_o = pso_live[qi2]
            j = (qi % 2) * 2 + hh
            for kt in range(NT):
                nc.tensor.matmul(
                    ps_o[:, j, 0:DH + 1],
                    pts[hh][:, kt * N + qi * P:kt * N + qi * P + P],
                    vx[:, kt, h, :],
                    start=(kt == 0), stop=(kt == NT - 1))

    def issue_norm(st, hp):
        # o[q, h*64:(h+1)*64] = num / den for the pair's 4 (qi&1, hh)
        # groups per bank: strided reciprocal + broadcast multiply
        bb, pts, vx, o_sb, pso_live, ot = st
        for qi2 in range(2):
            ps_o = pso_live.pop(qi2)
            rc = env["small_pool"].tile([P, 4, 1], FP32, tag="rc",
                                        name=f"rc_{bb}_{hp}_{qi2}")
            nc.vector.reciprocal(rc[:], ps_o[:, :, DH:DH + 1])
            nc.vector.tensor_tensor(
                o_sb[:, 2 * qi2:2 * qi2 + 2,
                     hp * 2 * DH:(hp + 1) * 2 * DH].rearrange(
                    "p q (c d) -> p q c d", d=DH),
                ps_o[:, :, 0:DH].rearrange("p (a c) d -> p a c d", c=2),
                rc.rearrange("p (a c) d -> p a c d", c=2).broadcast_to(
                    [P, 2, 2, DH]),
                op=MUL)
        for nt in range(NT):
            trans_q.dma_start_transpose(
                ot[:, hp, nt * P:(nt + 1) * P],
                o_sb[:, nt, hp * P:(hp + 1) * P])

    def load_x(b):
        x8 = None
        if CFG["qk_fp8"]:
            x8 = x8_pool.tile([P, ET, N], FP8, name=f"x8_{b}")
            dma_q.dma_start(
                x8[:], x8_d.ap()[b].rearrange("(c p) n -> p c n", p=P))
        xt = xt_pool.tile([P, ET, N], BF16, name=f"xt_{b}")
        dma_q.dma_start(xt[:], xT_d.ap()[b].rearrange("(c p) n -> p c n", p=P))
        return xt, x8

    env["next_x"] = load_x(0)
    if env["loop_iters"] == 1:
        env["load_persist2"]()
    for b in range(BPC):
        xt, x8 = env["next_x"]

        # ---- q/k projections, e-major output (qT[e_out, n])
        qt = qt_pool.tile([P, ET, N], BF16)
        ktl = kt_pool.tile([P, ET, N], BF16)
        for t in range(ET):
            for w_sb, b_sb, dst in ((wq_sb, bq_sb, qt), (wk_sb, bk_sb, ktl)):
                ps = ps_small.tile([P, N], FP32, tag="ps")
                if CFG["qk_fp8"]:
                    for c in range(ET // 2):
                        nc.tensor.matmul(
                            ps[:],
                            w_sb[:, 2 * c:2 * c + 2, t * P:(t + 1) * P],
                            x8[:, 2 * c:2 * c + 2, :],
                            start=(c == 0), stop=(c == ET // 2 - 1),
                            perf_mode=DR)
                else:
                    for kc in range(ET):
                        nc.tensor.matmul(
                            ps[:], w_sb[:, kc, t * P:(t + 1) * P],
                            xt[:, kc, :],
                            start=(kc == 0), stop=(kc == ET - 1))
                eng = CFG["evac_q"] if dst is qt else CFG["evac_k"]
                cut2 = CFG.get("evac_split", 0)
                if eng == "scalar":
                    nc.scalar.activation(
                        dst[:, t, :], ps[:], AF.Identity,
                        bias=b_sb[:, t:t + 1], scale=1.0)
                elif cut2:
                    nc.vector.tensor_scalar_add(
                        dst[:, t, 0:cut2], ps[:, 0:cut2],
                        b_sb[:, t:t + 1])
                    nc.scalar.activation(
                        dst[:, t, cut2:], ps[:, cut2:], AF.Identity,
                        bias=b_sb[:, t:t + 1], scale=1.0)
                else:
                    nc.vector.tensor_scalar_add(
                        dst[:, t, :], ps[:], b_sb[:, t:t + 1])

        # deferred last head pair of the previous batch: its P@V + norm
        # fill the batch-boundary pipeline bubble
        if env["pending_pv"] is not None:
            for hh in range(2):
                issue_o_head(env["pending_pv"], H // 2 - 1, hh)
            issue_norm(env["pending_pv"], H // 2 - 1)
            env["pending_pv"] = None

        # ---- v projection, n-major ([n, (h, d)]) + ones column
        hoist = CFG.get("hoist_s0", False)
        vx = vx_pool.tile([P, NT, H, DH + 1], BF16)
        nc.gpsimd.memset(vx[:, :, :, DH:DH + 1], 1.0)

        def issue_vproj():
            for nt in range(NT):
                ps = ps_small.tile([P, E], FP32, tag="ps")
                for kc in range(ET):
                    nc.tensor.matmul(
                        ps[:], xt[:, kc, nt * P:(nt + 1) * P],
                        wv_sb[:, kc, :],
                        start=(kc == 0), stop=(kc == ET - 1))
                nc.vector.tensor_add(
                    vx[:, nt, :, 0:DH],
                    ps.rearrange("p (h d) -> p h d", h=H),
                    bv_sb.rearrange("p (h d) -> p h d", h=H))
            if b + 1 < BPC:
                env["next_x"] = load_x(b + 1)

        if not hoist:
            issue_vproj()

        # ---- attention, head pairs (even head on PE rows 0-63, odd on
        # 64-127; scores land transposed [k, q])
        o_sb = o_pool.tile([P, NT, E], BF16)
        ot = ot_pool.tile([P, ET, N], BF16)
        adj_flat = adj_sb.rearrange("p c q -> p (c q)")
        st = (b, None, vx, o_sb, {}, ot)  # pts filled per pair

        def issue_scores(hp, ilv=None):
            t = hp
            pts = [pt_pool.tile([P, NT * N], BF16, tag="pt",
                                name=f"pt_{b}_{hp}_{i}")
                   for i in range(2)]
            for half in range(2):
                pss = [ps_big.tile([P, 2 * N], FP32, tag="scores",
                                   name=f"ss_{b}_{hp}_{half}_{i}")
                       for i in range(2)]
                for k2 in range(2):
                    kt = half * 2 + k2
                    for hh in range(2):
                        po = hh * DH
                        nc.tensor.matmul(
                            pss[hh][:, k2 * N:(k2 + 1) * N],
                            ktl[po:po + DH, t, kt * P:(kt + 1) * P],
                            qt[po:po + DH, t, :],
                            start=True, stop=True)
                lo = half * 2 * N
                fr = CFG["pool_mask_frac"]  # quarters of each chunk on Pool
                cut = lo + int((4 - fr) * N) // 2
                hi = (half + 1) * 2 * N
                for hh in range(2):
                    nc.scalar.activation(pts[hh][:, lo:hi], pss[hh][:],
                                         AF.Exp, scale=0.125)
                    nc.vector.tensor_tensor(
                        pts[hh][:, lo:cut], pts[hh][:, lo:cut],
                        adj_flat[:, lo:cut], op=MUL)
                    if fr:
                        nc.gpsimd.tensor_tensor(
                            pts[hh][:, cut:hi], pts[hh][:, cut:hi],
                            adj_flat[:, cut:hi], op=MUL)
                if ilv is not None:
                    issue_o_head(ilv, hp - 1, half)
                    if half == 1:
                        issue_norm(ilv, hp - 1)
            return pts

        def issue_final(args):
            bprev, o_prev, otprev = args
            for nt in range(NT):
                ps_f = ps_small.tile([P, E], FP32, tag="ps")
                for et in range(ET):
                    nc.tensor.matmul(
                        ps_f[:], otprev[:, et, nt * P:(nt + 1) * P],
                        wo_sb[:, et, :],
                        start=(et == 0), stop=False)
                nc.tensor.matmul(ps_f[:], ones1[:], bo1[:],
                                 start=False, stop=True)
                ob = out_pool.tile([P, E], FP32, tag="ob",
                                   name=f"ob_{bprev}_{nt}")
                nc.vector.tensor_copy(ob[:], ps_f[:])
                out_q.dma_start(
                    out_d.ap()[bprev, nt * P:(nt + 1) * P, :], ob[:])

        # interleave prev pair's o-stage between this pair's score chunks
        prev = None
        for hp in range(H // 2):
            cur = issue_scores(hp, ilv=(b, prev, vx, o_sb, st[4], ot)
                               if prev is not None else None)
            if hp == 0 and hoist:
                issue_vproj()
            if hp == 2 and env["pending_final"] is not None:
                issue_final(env["pending_final"])
                env["pending_final"] = None
            prev = cur
        if CFG["defer_pv"]:
            env["pending_pv"] = (b, prev, vx, o_sb, st[4], ot)
        else:
            last = (b, prev, vx, o_sb, st[4], ot)
            for hh in range(2):
                issue_o_head(last, H // 2 - 1, hh)
            issue_norm(last, H // 2 - 1)
        env["pending_final"] = (b, o_sb, ot)

    # drain the final deferred pair, then the last batch's output stage
    if env["pending_pv"] is not None:
        for hh in range(2):
            issue_o_head(env["pending_pv"], H // 2 - 1, hh)
        issue_norm(env["pending_pv"], H // 2 - 1)
        env["pending_pv"] = None
    bprev, o_prev, otprev = env["pending_final"]
    for nt in range(NT):
        ps_f = ps_small.tile([P, E], FP32, tag="ps")
        for et in range(ET):
            nc.tensor.matmul(
                ps_f[:], otprev[:, et, nt * P:(nt + 1) * P],
                wo_sb[:, et, :],
                start=(et == 0), stop=False)
        nc.tensor.matmul(ps_f[:], ones1[:], bo1[:], start=False, stop=True)
        ob = out_pool.tile([P, E], FP32, tag="ob", name=f"ob_{bprev}_{nt}")
        nc.vector.tensor_copy(ob[:], ps_f[:])
        out_q.dma_start(out_d.ap()[bprev, nt * P:(nt + 1) * P, :], ob[:])


_NC_CACHE = {}


def get_nc(loop_iters=1):
    if loop_iters not in _NC_CACHE:
        _NC_CACHE[loop_iters] = build_nc(loop_iters)
    return _NC_CACHE[loop_iters]


def prep_inputs(x, adj, Wq, Wk, Wv, bq, bk, bv, Wo, bo):
    """Host-side layout prep -> per-core input maps."""
    import ml_dtypes
    x = np.asarray(x, dtype=np.float32)
    F8 = ml_dtypes.float8_e4m3
    shared = {
        "WvT": np.ascontiguousarray(np.asarray(Wv, np.float32).T.astype(ml_dtypes.bfloat16)),
        "WoT": np.ascontiguousarray(np.asarray(Wo, np.float32).T.astype(ml_dtypes.bfloat16)),
        "bqT": np.ascontiguousarray(np.asarray(bq, np.float32).reshape(ET, P).T),
        "bkT": np.ascontiguousarray(np.asarray(bk, np.float32).reshape(ET, P).T),
        "bvB": np.ascontiguousarray(
            np.broadcast_to(np.asarray(bv, np.float32), (P, E))),
        "boB": np.ascontiguousarray(
            np.broadcast_to(np.asarray(bo, np.float32), (P, E))),
        "adjT": np.ascontiguousarray(
            np.asarray(adj).T.astype(ml_dtypes.bfloat16)),
        "bo16": np.ascontiguousarray(
            np.asarray(bo, np.float32).reshape(1, E).astype(
                ml_dtypes.bfloat16)),
    }
    if CFG["qk_fp8"]:
        shared["Wq8"] = np.ascontiguousarray(
            np.asarray(Wq, np.float32).T.astype(F8))
        shared["Wk8"] = np.ascontiguousarray(
            np.asarray(Wk, np.float32).T.astype(F8))
    else:
        shared["Wq8"] = np.ascontiguousarray(
            np.asarray(Wq, np.float32).T.astype(ml_dtypes.bfloat16))
        shared["Wk8"] = np.ascontiguousarray(
            np.asarray(Wk, np.float32).T.astype(ml_dtypes.bfloat16))
    in_maps = []
    for c in range(N_CORES):
        xs = x[c * BPC:(c + 1) * BPC]  # [BPC, N, E]
        m = dict(shared)
        xsT = xs.transpose(0, 2, 1)
        m["xT"] = np.ascontiguousarray(xsT.astype(ml_dtypes.bfloat16))
        if CFG["qk_fp8"]:
            m["xT8"] = np.ascontiguousarray(xsT.astype(F8))
        in_maps.append(m)
    return in_maps


def kernel(**inputs):
    import os
    # this container lacks the axon NTFF hook; never attempt tracing
    os.environ.setdefault("BASS_NEVER_TRACE", "1")
    nc = get_nc()
    in_maps = prep_inputs(**inputs)
    res = bass_utils.run_bass_kernel_spmd(
        nc, in_maps, core_ids=list(range(N_CORES)))
    return np.concatenate([r["out"] for r in res.results], axis=0)


# ---------------------------------------------------------------------------
# Benchmarking helpers (not used by the grading path). Runs the kernel with
# inputs resident on device, with the whole per-core computation repeated
# R times inside the NEFF (tc.For_i); HW time per iteration is estimated as
# (T(R2) - T(R1)) / (R2 - R1) to cancel the fixed dispatch overhead.
def _make_sharded_fn(nc):
    import jax
    from jax.sharding import Mesh, PartitionSpec, NamedSharding
    from jax.experimental.shard_map import shard_map
    from concourse import bass2jax

    bass2jax.install_neuronx_cc_hook()
    pid = nc.partition_id_tensor
    in_names, out_names, out_avals = [], [], []
    for alloc in nc.m.functions[0].allocations:
        if not isinstance(alloc, mybir.MemoryLocationSet):
            continue
        name = alloc.memorylocations[0].name
        if alloc.kind == "ExternalInput":
            if pid is None or name != pid.name:
                in_names.append(name)
        elif alloc.kind == "ExternalOutput":
            out_names.append(name)
            out_avals.append(jax.core.ShapedArray(
                tuple(alloc.tensor_shape), mybir.dt.np(alloc.dtype)))
    all_in_names = in_names + out_names
    if pid is not None:
        all_in_names.append(pid.name)

    def _body(*args):
        operands = list(args)
        if pid is not None:
            operands.append(bass2jax.partition_id_tensor())
        return tuple(bass2jax._bass_exec_p.bind(
            *operands,
            out_avals=tuple(out_avals),
            in_names=tuple(all_in_names),
            out_names=tuple(out_names),
            lowering_input_output_aliases=(),
            sim_require_finite=True,
            sim_require_nnan=True,
            nc=nc,
        ))

    devices = jax.devices()[:N_CORES]
    mesh = Mesh(np.asarray(devices), ("core",))
    spec = PartitionSpec("core")
    nin = len(in_names) + len(out_names)
    fn = jax.jit(
        shard_map(_body, mesh=mesh, in_specs=(spec,) * nin,
                  out_specs=(spec,) * len(out_names), check_rep=False),
        keep_unused=True,
    )
    return fn, in_names, out_names, out_avals, mesh, spec


def _time_nc(nc, in_maps, n_rep):
    import time
    import jax
    from jax.sharding import NamedSharding

    fn, in_names, out_names, out_avals, mesh, spec = _make_sharded_fn(nc)
    sh = NamedSharding(mesh, spec)
    args = []
    for name in in_names:
        args.append(jax.device_put(
            np.concatenate([m[name] for m in in_maps], axis=0), sh))
    for av in out_avals:
        args.append(jax.device_put(
            np.zeros((N_CORES * av.shape[0],) + av.shape[1:], av.dtype), sh))
    out = fn(*args)
    jax.block_until_ready(out)
    ts = []
    for _ in range(n_rep):
        t0 = time.perf_counter()
        out = fn(*args)
        jax.block_until_ready(out)
        ts.append(time.perf_counter() - t0)
    return min(ts), out


def benchmark(inputs, r1=256, r2=1024, n_rep=10):
    """Interleaved two-point measurement: the ~80 ms axon dispatch overhead
    (and its drift) cancels in the difference; device time dominates both."""
    import time
    import jax
    from jax.sharding import NamedSharding

    in_maps = prep_inputs(**inputs)

    def setup(r):
        nc = get_nc(r)
        fn, in_names, out_names, out_avals, mesh, spec = _make_sharded_fn(nc)
        sh = NamedSharding(mesh, spec)
        args = []
        for name in in_names:
            args.append(jax.device_put(
                np.concatenate([m[name] for m in in_maps], axis=0), sh))
        for av in out_avals:
            args.append(jax.device_put(
                np.zeros((N_CORES * av.shape[0],) + av.shape[1:], av.dtype),
                sh))
        out = fn(*args)
        jax.block_until_ready(out)
        return fn, args

    f1, a1 = setup(r1)
    f2, a2 = setup(r2)
    t1s, t2s = [], []
    for _ in range(n_rep):
        t0 = time.perf_counter()
        jax.block_until_ready(f1(*a1))
        t1s.append(time.perf_counter() - t0)
        t0 = time.perf_counter()
        jax.block_until_ready(f2(*a2))
        t2s.append(time.perf_counter() - t0)
    return (min(t2s) - min(t1s)) * 1e9 / (r2 - r1)
